# revision 19
# baseline (speedup 1.0000x reference)
"""MultiHeadAttention Trainium2 kernel, v3 (bf16, transposed PV).

Sharding: 2 batches x 4 head-groups over 8 cores. Core c handles batch
c//4 and heads 4*(c%4).. (256 of 1024 hidden features) over that
batch's 2048 tokens.

Per core:
  - Q/K/V projections in bf16 (feature-major Q^T/K^T for scores,
    token-major V-hat with a ones-column per head for the softmax
    denominators).
  - scoresT = K Q^T per head (keys on partitions); exp on ScalarE with
    the attention-mask column as per-partition bias, probs bf16 into a
    per-panel [128, kt, 1024] buffer (3 panels in flight).
  - PV transposed: the probs tile is the stationary operand, V-hat
    moving, giving token-major attn [128 tokens, 65] accumulated over
    key tiles, 4 q-tiles packed per psum bank as one accumulation
    group.  Normalization is a per-partition tensor_scalar divide by
    the denominator column.
  - attn panels transposed back to feature-major via the XBAR
    DMA-transpose and fed to the output projection; psum copies and
    stores alternate DVE/Pool and sync/gpsimd.

ScalarE exp is ~133us and PE ~139us busy; to keep both saturated the
emission interleaves every non-scores PE work unit (projection
n-tiles, V tiles, PV groups, out-proj tiles) between individual
scores key-tiles as "filler", hand-scheduled per panel.

Host sums 4 partials per batch and adds bo.
"""

import numpy as np
import ml_dtypes

import concourse.bass as bass
import concourse.tile as tile
from concourse import bacc, mybir
from concourse.bass import ts

BF16 = mybir.dt.bfloat16
F32 = mybir.dt.float32
NPBF = ml_dtypes.bfloat16

B, S, H = 2, 2048, 1024
NHEAD, D = 16, 64
HPC = 4                   # heads per core
F = HPC * D               # 256 features per core
KC = H // 128             # 8 contraction chunks
VW = HPC * (D + 1)        # 260
SCALE = 1.0 / np.sqrt(D)

_CACHE = {}


def _build_nc(s=S):
    tok = s                    # tokens per core (one batch)
    tt_n = tok // 128          # 16 token tiles
    nt_n = tok // 512          # 4 proj n-tiles
    kt_n = s // 128            # 16 key tiles
    qhw = 1024                 # q-panel width
    qh_n = s // qhw            # 2
    nqt = qhw // 512           # 2
    qt_n = qhw // 128          # 8 q-tiles per panel

    nc = bacc.Bacc("TRN2", target_bir_lowering=False, debug=False)

    xT = nc.dram_tensor("xT", [H, tok], BF16, kind="ExternalInput")
    wq = nc.dram_tensor("wq", [128, KC, 2, 128], BF16, kind="ExternalInput")
    wk = nc.dram_tensor("wk", [128, KC, 2, 128], BF16, kind="ExternalInput")
    wv = nc.dram_tensor("wv", [128, KC, VW], BF16, kind="ExternalInput")
    bq = nc.dram_tensor("bq", [128, 2], F32, kind="ExternalInput")
    bk = nc.dram_tensor("bk", [128, 2], F32, kind="ExternalInput")
    bvh = nc.dram_tensor("bvh", [128, VW], F32, kind="ExternalInput")
    mk = nc.dram_tensor("mk", [128, kt_n], F32, kind="ExternalInput")
    wo = nc.dram_tensor("wo", [128, 2, H], BF16, kind="ExternalInput")
    out = nc.dram_tensor("out", [tok, H], BF16, kind="ExternalOutput")

    with tile.TileContext(nc) as tc:
        with (
            tc.tile_pool(name="consts", bufs=1) as consts,
            tc.tile_pool(name="big", bufs=1) as big,
            tc.tile_pool(name="ptp", bufs=3) as ptp,
            tc.tile_pool(name="anp", bufs=1) as anp,
            tc.tile_pool(name="stage", bufs=4) as stage,
        ):
            xT_sb = big.tile([128, KC, tok], BF16)
            qT_sb = big.tile([128, 2, tok], BF16)
            kT_sb = big.tile([128, 2, tok], BF16)
            vh_sb = big.tile([128, tt_n, VW], BF16)
            aT_sb = big.tile([128, 2, tok], BF16)
            wq_sb = consts.tile([128, KC, 2, 128], BF16)
            wk_sb = consts.tile([128, KC, 2, 128], BF16)
            wv_sb = consts.tile([128, KC, VW], BF16)
            wo_sb = consts.tile([128, 2, H], BF16)
            bq_sb = consts.tile([128, 2], F32)
            bk_sb = consts.tile([128, 2], F32)
            bvh_sb = consts.tile([128, VW], F32)
            mk_sb = consts.tile([128, kt_n], F32)

            # xT chunks pipelined in k-order over all three queues so the
            # chunk-outer K projection never waits; weights interleave on
            # the scalar queue (idle before the first exp)
            # xT in column-halves: the first q-panel's K/Q projections only
            # touch tokens 0-1023, so those arrive in ~2.6us over 3 queues
            hw_ = tok // 2
            lq = [nc.sync, nc.gpsimd, nc.scalar]

            def ldx(i, k, half):
                cs = slice(half * hw_, (half + 1) * hw_)
                lq[i % 3].dma_start(out=xT_sb[:, k, cs],
                                    in_=xT[k * 128:(k + 1) * 128, cs])

            nc.sync.dma_start(out=wk_sb[:, 0, :, :], in_=wk[:, 0, :, :])
            nc.scalar.dma_start(out=wk_sb[:, 1:KC, :, :], in_=wk[:, 1:KC, :, :])
            nc.scalar.dma_start(out=wq_sb, in_=wq[:, :, :, :])
            for k in range(KC):
                ldx(k + k // 2, k, 0)       # sync/gpsimd only
            for k in range(KC):
                ldx(k, k, 1)
            nc.gpsimd.dma_start(out=wv_sb, in_=wv[:, :, :])
            nc.gpsimd.dma_start(out=wo_sb, in_=wo[:, :, :])
            nc.sync.dma_start(out=bq_sb, in_=bq[:, :])
            nc.sync.dma_start(out=bk_sb, in_=bk[:, :])
            nc.sync.dma_start(out=bvh_sb, in_=bvh[:, :])
            nc.sync.dma_start(out=mk_sb, in_=mk[:, :])

            with (
                tc.tile_pool(name="ps_s", bufs=2, space="PSUM") as pss,
                tc.tile_pool(name="ps_av", bufs=2, space="PSUM") as psav,
                tc.tile_pool(name="ps_p", bufs=2, space="PSUM") as psp,
            ):
                pts = {}
                an = {0: {}, 1: {}}
                state = {"oi": 0, "pool_o": psp}

                # ---- filler work units (generators emitting one unit) ----
                def u_proj(w_sb, t_sb, b_sb, fg, nt):
                    ps = state["pool_o"].tile([128, 512], F32, tag="proj",
                                              name=f"pj_{id(w_sb)}_{fg}_{nt}")
                    for k in range(KC):
                        nc.tensor.matmul(
                            ps, w_sb[:, k, fg, :], xT_sb[:, k, ts(nt, 512)],
                            start=(k == 0), stop=(k == KC - 1),
                        )
                    nc.vector.tensor_scalar_add(
                        t_sb[:, fg, ts(nt, 512)], ps, b_sb[:, fg:fg + 1])

                def u_vproj(tt):
                    vps = state["pool_o"].tile([128, 512], F32, tag="proj",
                                               name=f"vps_{tt}")
                    for k in range(KC):
                        nc.tensor.matmul(
                            vps[:, 0:VW], xT_sb[:, k, ts(tt, 128)],
                            wv_sb[:, k, :],
                            start=(k == 0), stop=(k == KC - 1),
                        )
                    nc.vector.tensor_add(vh_sb[:, tt, :], vps[:, 0:VW], bvh_sb)

                def u_av_group(qh, h, g):
                    # 4 q-tiles of PV in one accumulation group / psum bank
                    fg, hh = h // 2, h % 2
                    vc = slice(h * (D + 1), (h + 1) * (D + 1))
                    pt = pts[(qh, h)]
                    av4 = psav.tile([128, 4, 65], F32, tag="av",
                                    name=f"av_{qh}_{h}_{g}")
                    for j in range(4):
                        qt = g * 4 + j
                        for kt in range(kt_n):
                            nc.tensor.matmul(
                                av4[:, j, :],
                                pt[:, kt, ts(qt, 128)],
                                vh_sb[:, kt, vc],
                                start=(j == 0 and kt == 0),
                                stop=(j == 3 and kt == kt_n - 1),
                                skip_group_check=True,
                            )
                    for j in range(4):
                        qt = g * 4 + j
                        key = (qt, fg)
                        if key not in an[qh]:
                            an[qh][key] = anp.tile(
                                [128, 128], BF16, tag=f"an{qt}_{fg}",
                                name=f"an_{qh}_{qt}_{fg}")
                        rc = stage.tile([128, 1], F32, tag="rc")
                        nc.vector.reciprocal(rc, av4[:, j, 64:65])
                        nc.vector.tensor_scalar_mul(
                            an[qh][key][:, hh * 64:(hh + 1) * 64],
                            av4[:, j, 0:64], rc)
                    if h == 3:
                        for j in range(4):
                            qt = g * 4 + j
                            g0 = qh * qhw + qt * 128
                            for fg2 in range(2):
                                nc.sync.dma_start(
                                    out=aT_sb[:, fg2, g0:g0 + 128],
                                    in_=an[qh][(qt, fg2)],
                                    transpose=True,
                                )

                def u_outproj(qh, tt, copy_eng="dve"):
                    g0 = qh * qhw + tt * 128
                    for no in range(2):
                        ops = state["pool_o"].tile([128, 512], F32, tag="proj",
                                                   name=f"op_{qh}_{tt}_{no}")
                        for fg in range(2):
                            nc.tensor.matmul(
                                ops, aT_sb[:, fg, g0:g0 + 128],
                                wo_sb[:, fg, ts(no, 512)],
                                start=(fg == 0), stop=(fg == 1),
                            )
                        st = stage.tile([128, 512], BF16, tag="st")
                        oi = state["oi"]
                        # gpsimd cannot read PSUM on hw; in the tail all
                        # copies go to ScalarE (idle once the exps finish)
                        # while DVE handles the normalize divides
                        if copy_eng == "act":
                            nc.scalar.copy(st, ops)
                        else:
                            nc.vector.tensor_copy(st, ops)
                        if oi % 2 == 0:
                            nc.sync.dma_start(
                                out=out[g0:g0 + 128, no * 512:(no + 1) * 512],
                                in_=st)
                        else:
                            nc.gpsimd.dma_start(
                                out=out[g0:g0 + 128, no * 512:(no + 1) * 512],
                                in_=st)
                        state["oi"] = oi + 1

                # ---- scores panel with interleaved filler ----
                def scores_panel(qh, h, filler):
                    q0 = qh * qhw
                    fg, hh = h // 2, h % 2
                    hr = slice(hh * 64, (hh + 1) * 64)
                    pt = ptp.tile([128, kt_n, qhw], BF16, tag="pt",
                                  name=f"pt_{qh}_{h}")
                    pts[(qh, h)] = pt
                    # filler: (slot, thunk) pairs run after that kt's exp;
                    # bare thunks are spread evenly over the 16 slots
                    bare = [f for f in filler if not isinstance(f, tuple)]
                    placed = sorted(
                        [f for f in filler if isinstance(f, tuple)]
                        + [((i + 1) * kt_n // (len(bare) + 1), f)
                           for i, f in enumerate(bare)],
                        key=lambda p: p[0])
                    fi = 0
                    for kt in range(kt_n):
                        sps = pss.tile([128, qhw], F32, tag="s")
                        for qt in range(nqt):
                            nc.tensor.matmul(
                                sps[:, ts(qt, 512)],
                                kT_sb[hr, fg, kt * 128:(kt + 1) * 128],
                                qT_sb[hr, fg,
                                      q0 + qt * 512:q0 + (qt + 1) * 512],
                                start=True, stop=True,
                            )
                        nc.scalar.activation(
                            out=pt[:, kt, :], in_=sps,
                            func=mybir.ActivationFunctionType.Exp,
                            bias=mk_sb[:, kt:kt + 1],
                            scale=float(SCALE),
                        )
                        while fi < len(placed) and placed[fi][0] <= kt:
                            placed[fi][1]()
                            fi += 1
                    while fi < len(placed):
                        placed[fi][1]()
                        fi += 1

                P = lambda *a: (lambda: u_proj(*a))
                V = lambda tt: (lambda: u_vproj(tt))
                AV = lambda qh, h, g: (lambda: u_av_group(qh, h, g))
                OP = lambda qh, tt: (lambda: u_outproj(qh, tt))

                # prelude: only what scores(0,0) kt0-3 needs — K n-tile 0
                # and the Q panel halves; remaining K n-tiles are filler
                # positioned just ahead of the key tiles that need them
                u_proj(wk_sb, kT_sb, bk_sb, 0, 0)
                u_proj(wq_sb, qT_sb, bq_sb, 0, 0)
                u_proj(wq_sb, qT_sb, bq_sb, 0, 1)

                scores_panel(0, 0, [(0, P(wk_sb, kT_sb, bk_sb, 0, 1)),
                                    (4, P(wk_sb, kT_sb, bk_sb, 0, 2)),
                                    (8, P(wk_sb, kT_sb, bk_sb, 0, 3))])
                scores_panel(0, 1, [P(wk_sb, kT_sb, bk_sb, 1, nt)
                                    for nt in range(nt_n)]
                             + [P(wq_sb, qT_sb, bq_sb, 1, 0),
                                P(wq_sb, qT_sb, bq_sb, 1, 1)])
                scores_panel(0, 2, [(tt * 3 // 4, V(tt)) for tt in range(16)]
                             + [(12, AV(0, 0, 0)), (14, AV(0, 0, 1))])
                scores_panel(0, 3, [(0, P(wq_sb, qT_sb, bq_sb, 0, 2)),
                                    (4, P(wq_sb, qT_sb, bq_sb, 0, 3)),
                                    (10, AV(0, 1, 0)), (13, AV(0, 1, 1))])
                scores_panel(1, 0, [(0, P(wq_sb, qT_sb, bq_sb, 1, 2)),
                                    (4, P(wq_sb, qT_sb, bq_sb, 1, 3))])
                scores_panel(1, 1, [(0, AV(0, 2, 0)), (2, AV(0, 2, 1)),
                                    (8, AV(0, 3, 0)), (11, AV(0, 3, 1))])
                scores_panel(1, 2, [OP(0, tt) for tt in range(qt_n)]
                             + [(12, AV(1, 0, 0)), (14, AV(1, 0, 1))])
                scores_panel(1, 3, [(0, AV(1, 1, 0)), (2, AV(1, 1, 1)),
                                    (6, AV(1, 2, 0)), (9, AV(1, 2, 1))])
                # tail: both PV groups of the last panel accumulate
                # kt-interleaved, so only the kt15 matmuls trail the final
                # exp; then divide -> transpose -> out-proj per q-tile
                h, fg, hh = 3, 1, 1
                vc = slice(h * (D + 1), (h + 1) * (D + 1))
                pt = pts[(1, 3)]
                avg = [psav.tile([128, 4, 65], F32, tag="av",
                                 name=f"avt_{g}") for g in range(2)]
                for kt in range(kt_n):
                    for g in range(2):
                        for j in range(4):
                            qt = g * 4 + j
                            nc.tensor.matmul(
                                avg[g][:, j, :], pt[:, kt, ts(qt, 128)],
                                vh_sb[:, kt, vc],
                                start=(kt == 0 and j == 0),
                                stop=(kt == kt_n - 1 and j == 3),
                                skip_group_check=True,
                            )
                for g in range(2):
                    for j in range(4):
                        qt = g * 4 + j
                        rc = stage.tile([128, 1], F32, tag="rc")
                        nc.vector.reciprocal(rc, avg[g][:, j, 64:65])
                        nc.vector.tensor_scalar_mul(
                            an[1][(qt, fg)][:, hh * 64:(hh + 1) * 64],
                            avg[g][:, j, 0:64], rc)
                        g0 = qhw + qt * 128
                        for fg2 in range(2):
                            nc.sync.dma_start(
                                out=aT_sb[:, fg2, g0:g0 + 128],
                                in_=an[1][(qt, fg2)],
                                transpose=True,
                            )
                        u_outproj(1, qt, copy_eng="act")

    nc.compile()
    return nc


def _prep_inputs(x, attention_mask, Wq, bq, Wk, bk, Wv, bv, Wo, bo, s=S):
    kt_n = s // 128
    x = np.asarray(x, dtype=np.float32)
    Wq = np.asarray(Wq, np.float32)
    Wk = np.asarray(Wk, np.float32)
    Wv = np.asarray(Wv, np.float32)
    Wo = np.asarray(Wo, np.float32)
    bqf = np.asarray(bq, np.float32)
    bkf = np.asarray(bk, np.float32)
    bvf = np.asarray(bv, np.float32)
    mask = np.asarray(attention_mask, np.float32).reshape(B, s)

    xT_b = [np.ascontiguousarray(x[b].T.astype(NPBF)) for b in range(B)]
    mk_b = [np.ascontiguousarray(mask[b].reshape(kt_n, 128).T)
            for b in range(B)]

    in_maps = []
    for c in range(8):
        b = c // 4
        hg = c % 4
        r = slice(hg * F, (hg + 1) * F)
        # [128, KC, 2, 128]: partition p, chunk k -> W row k*128+p
        wqc = np.ascontiguousarray(
            Wq[r].T.astype(NPBF).reshape(KC, 128, 2, 128).transpose(1, 0, 2, 3))
        wkc = np.ascontiguousarray(
            Wk[r].T.astype(NPBF).reshape(KC, 128, 2, 128).transpose(1, 0, 2, 3))
        wvc = np.zeros((H, VW), dtype=NPBF)
        bvh1 = np.empty((1, VW), dtype=np.float32)
        for h in range(HPC):
            rh = slice(hg * F + h * D, hg * F + (h + 1) * D)
            wvc[:, h * (D + 1):h * (D + 1) + D] = Wv[rh].T.astype(NPBF)
            bvh1[0, h * (D + 1):h * (D + 1) + D] = bvf[rh]
            bvh1[0, h * (D + 1) + D] = 1.0
        wvc = np.ascontiguousarray(
            wvc.reshape(KC, 128, VW).transpose(1, 0, 2))
        woc = np.ascontiguousarray(
            Wo[:, r].T.astype(NPBF).reshape(2, 128, H).transpose(1, 0, 2))
        in_maps.append({
            "xT": xT_b[b],
            "wq": wqc,
            "wk": wkc,
            "wv": wvc,
            "bq": np.ascontiguousarray(bqf[r].reshape(2, 128).T),
            "bk": np.ascontiguousarray(bkf[r].reshape(2, 128).T),
            "bvh": np.ascontiguousarray(np.broadcast_to(bvh1, (128, VW))),
            "mk": mk_b[b],
            "wo": woc,
        })
    return in_maps


def kernel(x, attention_mask, Wq, bq, Wk, bk, Wv, bv, Wo, bo, _want_results=False):
    from concourse.bass_utils import run_bass_kernel_spmd

    if "nc" not in _CACHE:
        _CACHE["nc"] = _build_nc()
    nc = _CACHE["nc"]
    in_maps = _prep_inputs(x, attention_mask, Wq, bq, Wk, bk, Wv, bv, Wo, bo)
    res = run_bass_kernel_spmd(nc, in_maps, core_ids=list(range(8)))
    acc = np.zeros((B, S, H), dtype=np.float32)
    for c, r in enumerate(res.results):
        acc[c // 4] += r["out"].astype(np.float32)
    acc += np.asarray(bo, dtype=np.float32)[None, None, :]
    if _want_results:
        return acc, res
    return acc


# revision 38
# speedup vs baseline: 1.0206x; 1.0206x over previous
"""MultiHeadAttention Trainium2 kernel, v3 (bf16, transposed PV).

Sharding: 2 batches x 4 head-groups over 8 cores. Core c handles batch
c//4 and heads 4*(c%4).. (256 of 1024 hidden features) over that
batch's 2048 tokens.

Per core:
  - Q/K/V projections in bf16 (feature-major Q^T/K^T for scores,
    token-major V-hat with a ones-column per head for the softmax
    denominators).
  - scoresT = K Q^T per head (keys on partitions); exp on ScalarE with
    the attention-mask column as per-partition bias, probs bf16 into a
    per-panel [128, kt, 1024] buffer (3 panels in flight).
  - PV transposed: the probs tile is the stationary operand, V-hat
    moving, giving token-major attn [128 tokens, 65] accumulated over
    key tiles, 4 q-tiles packed per psum bank as one accumulation
    group.  Normalization is a per-partition tensor_scalar divide by
    the denominator column.
  - attn panels transposed back to feature-major via the XBAR
    DMA-transpose and fed to the output projection; psum copies and
    stores alternate DVE/Pool and sync/gpsimd.

ScalarE exp is ~133us and PE ~139us busy; to keep both saturated the
emission interleaves every non-scores PE work unit (projection
n-tiles, V tiles, PV groups, out-proj tiles) between individual
scores key-tiles as "filler", hand-scheduled per panel.

Host sums 4 partials per batch and adds bo.
"""

import numpy as np
import ml_dtypes

import concourse.bass as bass
import concourse.tile as tile
from concourse import bacc, mybir
from concourse.bass import ts

BF16 = mybir.dt.bfloat16
F32 = mybir.dt.float32
NPBF = ml_dtypes.bfloat16

B, S, H = 2, 2048, 1024
NHEAD, D = 16, 64
HPC = 4                   # heads per core
F = HPC * D               # 256 features per core
KC = H // 128             # 8 contraction chunks
VW = HPC * (D + 1)        # 260
SCALE = 1.0 / np.sqrt(D)
SCH_A = (1 << 23) / np.log(2.0)        # Schraudolph fast-exp constants
SCH_B = 127.0 * (1 << 23) - 498000.0   # calibrated for trunc-cast, x in [-2,2]

_CACHE = {}


def _build_nc(s=S):
    tok = s                    # tokens per core (one batch)
    tt_n = tok // 128          # 16 token tiles
    nt_n = tok // 512          # 4 proj n-tiles
    kt_n = s // 128            # 16 key tiles
    qhw = 1024                 # q-panel width
    qh_n = s // qhw            # 2
    nqt = qhw // 512           # 2
    qt_n = qhw // 128          # 8 q-tiles per panel

    nc = bacc.Bacc("TRN2", target_bir_lowering=False, debug=False)

    xT = nc.dram_tensor("xT", [H, tok], BF16, kind="ExternalInput")
    wq = nc.dram_tensor("wq", [128, KC, 2, 128], BF16, kind="ExternalInput")
    wk = nc.dram_tensor("wk", [128, KC, 2, 128], BF16, kind="ExternalInput")
    wv = nc.dram_tensor("wv", [128, KC, VW], BF16, kind="ExternalInput")
    bq = nc.dram_tensor("bq", [128, 2], F32, kind="ExternalInput")
    bk = nc.dram_tensor("bk", [128, 2], F32, kind="ExternalInput")
    bvh = nc.dram_tensor("bvh", [128, VW], F32, kind="ExternalInput")
    mk = nc.dram_tensor("mk", [128, kt_n], F32, kind="ExternalInput")
    mk2 = nc.dram_tensor("mk2", [128, kt_n], F32, kind="ExternalInput")
    wo = nc.dram_tensor("wo", [128, 2, H], BF16, kind="ExternalInput")
    out = nc.dram_tensor("out", [tok, H], BF16, kind="ExternalOutput")

    with tile.TileContext(nc) as tc:
        with (
            tc.tile_pool(name="consts", bufs=1) as consts,
            tc.tile_pool(name="big", bufs=1) as big,
            tc.tile_pool(name="ptp", bufs=3) as ptp,
            tc.tile_pool(name="anp", bufs=1) as anp,
            tc.tile_pool(name="itp", bufs=2) as itp,
            tc.tile_pool(name="stage", bufs=4) as stage,
        ):
            xT_sb = big.tile([128, KC, tok], BF16)
            qT_sb = big.tile([128, 2, tok], BF16)
            kT_sb = big.tile([128, 2, tok], BF16)
            vh_sb = big.tile([128, tt_n, VW], BF16)
            aT_sb = big.tile([128, 2, tok], BF16)
            wq_sb = consts.tile([128, KC, 2, 128], BF16)
            wk_sb = consts.tile([128, KC, 2, 128], BF16)
            wv_sb = consts.tile([128, KC, VW], BF16)
            wo_sb = consts.tile([128, 2, H], BF16)
            bq_sb = consts.tile([128, 2], F32)
            bk_sb = consts.tile([128, 2], F32)
            bvh_sb = consts.tile([128, VW], F32)
            mk_sb = consts.tile([128, kt_n], F32)
            mk2_sb = consts.tile([128, kt_n], F32)

            # xT chunks pipelined in k-order over all three queues so the
            # chunk-outer K projection never waits; weights interleave on
            # the scalar queue (idle before the first exp)
            # xT in column-halves: the first q-panel's K/Q projections only
            # touch tokens 0-1023, so those arrive in ~2.6us over 3 queues
            hw_ = tok // 2
            lq = [nc.sync, nc.gpsimd, nc.scalar]

            def ldx(i, k, half):
                cs = slice(half * hw_, (half + 1) * hw_)
                lq[i % 3].dma_start(out=xT_sb[:, k, cs],
                                    in_=xT[k * 128:(k + 1) * 128, cs])

            nc.sync.dma_start(out=wk_sb[:, 0, :, :], in_=wk[:, 0, :, :])
            nc.scalar.dma_start(out=wk_sb[:, 1:KC, :, :], in_=wk[:, 1:KC, :, :])
            nc.scalar.dma_start(out=wq_sb, in_=wq[:, :, :, :])
            for k in range(KC):
                ldx(k + k // 2, k, 0)       # sync/gpsimd only
            for k in range(KC):
                ldx(k, k, 1)
            nc.gpsimd.dma_start(out=wv_sb, in_=wv[:, :, :])
            nc.gpsimd.dma_start(out=wo_sb, in_=wo[:, :, :])
            nc.sync.dma_start(out=bq_sb, in_=bq[:, :])
            nc.sync.dma_start(out=bk_sb, in_=bk[:, :])
            nc.sync.dma_start(out=bvh_sb, in_=bvh[:, :])
            nc.sync.dma_start(out=mk_sb, in_=mk[:, :])
            nc.sync.dma_start(out=mk2_sb, in_=mk2[:, :])

            with (
                tc.tile_pool(name="ps_s", bufs=2, space="PSUM") as pss,
                tc.tile_pool(name="ps_av", bufs=2, space="PSUM") as psav,
                tc.tile_pool(name="ps_p", bufs=2, space="PSUM") as psp,
            ):
                pts = {}
                an = {0: {}, 1: {}}
                state = {"oi": 0, "pool_o": psp}

                # ---- filler work units (generators emitting one unit) ----
                def u_proj(w_sb, t_sb, b_sb, fg, nt):
                    ps = state["pool_o"].tile([128, 512], F32, tag="proj",
                                              name=f"pj_{id(w_sb)}_{fg}_{nt}")
                    for k in range(KC):
                        nc.tensor.matmul(
                            ps, w_sb[:, k, fg, :], xT_sb[:, k, ts(nt, 512)],
                            start=(k == 0), stop=(k == KC - 1),
                        )
                    nc.vector.tensor_scalar_add(
                        t_sb[:, fg, ts(nt, 512)], ps, b_sb[:, fg:fg + 1])

                def u_vproj(tt):
                    vps = state["pool_o"].tile([128, 512], F32, tag="proj",
                                               name=f"vps_{tt}")
                    for k in range(KC):
                        nc.tensor.matmul(
                            vps[:, 0:VW], xT_sb[:, k, ts(tt, 128)],
                            wv_sb[:, k, :],
                            start=(k == 0), stop=(k == KC - 1),
                        )
                    nc.vector.tensor_add(vh_sb[:, tt, :], vps[:, 0:VW], bvh_sb)

                def u_av_group(qh, h, g):
                    # 4 q-tiles of PV in one accumulation group / psum bank
                    fg, hh = h // 2, h % 2
                    vc = slice(h * (D + 1), (h + 1) * (D + 1))
                    pt = pts[(qh, h)]
                    av4 = psav.tile([128, 4, 65], F32, tag="av",
                                    name=f"av_{qh}_{h}_{g}")
                    for j in range(4):
                        qt = g * 4 + j
                        for kt in range(kt_n):
                            nc.tensor.matmul(
                                av4[:, j, :],
                                pt[:, kt, ts(qt, 128)],
                                vh_sb[:, kt, vc],
                                start=(j == 0 and kt == 0),
                                stop=(j == 3 and kt == kt_n - 1),
                                skip_group_check=True,
                            )
                    for j in range(4):
                        qt = g * 4 + j
                        key = (qt, fg)
                        if key not in an[qh]:
                            an[qh][key] = anp.tile(
                                [128, 128], BF16, tag=f"an{qt}_{fg}",
                                name=f"an_{qh}_{qt}_{fg}")
                        rc = stage.tile([128, 1], F32, tag="rc")
                        nc.vector.reciprocal(rc, av4[:, j, 64:65])
                        nc.vector.tensor_scalar_mul(
                            an[qh][key][:, hh * 64:(hh + 1) * 64],
                            av4[:, j, 0:64], rc)
                    if h == 3:
                        for j in range(4):
                            qt = g * 4 + j
                            g0 = qh * qhw + qt * 128
                            for fg2 in range(2):
                                nc.sync.dma_start(
                                    out=aT_sb[:, fg2, g0:g0 + 128],
                                    in_=an[qh][(qt, fg2)],
                                    transpose=True,
                                )

                def u_outproj(qh, tt, copy_eng="dve"):
                    g0 = qh * qhw + tt * 128
                    for no in range(2):
                        ops = state["pool_o"].tile([128, 512], F32, tag="proj",
                                                   name=f"op_{qh}_{tt}_{no}")
                        for fg in range(2):
                            nc.tensor.matmul(
                                ops, aT_sb[:, fg, g0:g0 + 128],
                                wo_sb[:, fg, ts(no, 512)],
                                start=(fg == 0), stop=(fg == 1),
                            )
                        st = stage.tile([128, 512], BF16, tag="st")
                        oi = state["oi"]
                        # gpsimd cannot read PSUM on hw; in the tail all
                        # copies go to ScalarE (idle once the exps finish)
                        # while DVE handles the normalize divides
                        if copy_eng == "act":
                            # tail: ScalarE and DVE alternate copies, sync
                            # queue is kept for transposes, stores on gpsimd
                            if oi % 2 == 0:
                                nc.scalar.copy(st, ops)
                            else:
                                nc.vector.tensor_copy(st, ops)
                            nc.gpsimd.dma_start(
                                out=out[g0:g0 + 128, no * 512:(no + 1) * 512],
                                in_=st)
                        else:
                            nc.vector.tensor_copy(st, ops)
                            (nc.sync if oi % 2 == 0 else nc.gpsimd).dma_start(
                                out=out[g0:g0 + 128, no * 512:(no + 1) * 512],
                                in_=st)
                        state["oi"] = oi + 1

                # ---- scores panel with interleaved filler ----
                # dve_kts: key tiles whose exp runs on DVE via the
                # Schraudolph bit-trick (bitcast(int32(A*x + B)) ~ exp(x),
                # ~1.7% rms on those probs) to shorten the ScalarE stream
                def scores_panel(qh, h, filler, dve_kts=()):
                    q0 = qh * qhw
                    fg, hh = h // 2, h % 2
                    hr = slice(hh * 64, (hh + 1) * 64)
                    pt = ptp.tile([128, kt_n, qhw], BF16, tag="pt",
                                  name=f"pt_{qh}_{h}")
                    pts[(qh, h)] = pt
                    # filler: (slot, thunk) pairs run after that kt's exp;
                    # bare thunks are spread evenly over the 16 slots
                    bare = [f for f in filler if not isinstance(f, tuple)]
                    placed = sorted(
                        [f for f in filler if isinstance(f, tuple)]
                        + [((i + 1) * kt_n // (len(bare) + 1), f)
                           for i, f in enumerate(bare)],
                        key=lambda p: p[0])
                    fi = 0
                    for kt in range(kt_n):
                        sps = pss.tile([128, qhw], F32, tag="s")
                        for qt in range(nqt):
                            nc.tensor.matmul(
                                sps[:, ts(qt, 512)],
                                kT_sb[hr, fg, kt * 128:(kt + 1) * 128],
                                qT_sb[hr, fg,
                                      q0 + qt * 512:q0 + (qt + 1) * 512],
                                start=True, stop=True,
                            )
                        if kt in dve_kts:
                            it = itp.tile([128, qhw], mybir.dt.int32,
                                          tag="it")
                            nc.vector.tensor_scalar(
                                out=it, in0=sps,
                                scalar1=float(SCH_A * SCALE),
                                scalar2=mk2_sb[:, kt:kt + 1],
                                op0=mybir.AluOpType.mult,
                                op1=mybir.AluOpType.add,
                            )
                            # int tile is SBUF, so Pool may do the bitcast
                            # copy; DVE only holds the scores psum briefly
                            nc.gpsimd.tensor_copy(
                                pt[:, kt, :], it.bitcast(F32))
                        else:
                            nc.scalar.activation(
                                out=pt[:, kt, :], in_=sps,
                                func=mybir.ActivationFunctionType.Exp,
                                bias=mk_sb[:, kt:kt + 1],
                                scale=float(SCALE),
                            )
                        while fi < len(placed) and placed[fi][0] <= kt:
                            placed[fi][1]()
                            fi += 1
                    while fi < len(placed):
                        placed[fi][1]()
                        fi += 1

                P = lambda *a: (lambda: u_proj(*a))
                V = lambda tt: (lambda: u_vproj(tt))
                AV = lambda qh, h, g: (lambda: u_av_group(qh, h, g))
                OP = lambda qh, tt: (lambda: u_outproj(qh, tt))

                # prelude: only what scores(0,0) kt0-3 needs — K n-tile 0
                # and the Q panel halves; remaining K n-tiles are filler
                # positioned just ahead of the key tiles that need them
                u_proj(wk_sb, kT_sb, bk_sb, 0, 0)
                u_proj(wq_sb, qT_sb, bq_sb, 0, 0)
                u_proj(wq_sb, qT_sb, bq_sb, 0, 1)

                scores_panel(0, 0, [(0, P(wk_sb, kT_sb, bk_sb, 0, 1)),
                                    (4, P(wk_sb, kT_sb, bk_sb, 0, 2)),
                                    (8, P(wk_sb, kT_sb, bk_sb, 0, 3))])
                scores_panel(0, 1, [P(wk_sb, kT_sb, bk_sb, 1, nt)
                                    for nt in range(nt_n)]
                             + [P(wq_sb, qT_sb, bq_sb, 1, 0),
                                P(wq_sb, qT_sb, bq_sb, 1, 1)]
                             + [(11 + tt, V(tt)) for tt in range(4)])
                scores_panel(0, 2, [(tt - 4, V(tt)) for tt in range(4, 16)]
                             + [(13, AV(0, 0, 0)), (15, AV(0, 0, 1))])
                scores_panel(0, 3, [(0, P(wq_sb, qT_sb, bq_sb, 0, 2)),
                                    (4, P(wq_sb, qT_sb, bq_sb, 0, 3)),
                                    (10, AV(0, 1, 0)), (13, AV(0, 1, 1))])
                scores_panel(1, 0, [(0, P(wq_sb, qT_sb, bq_sb, 1, 2)),
                                    (4, P(wq_sb, qT_sb, bq_sb, 1, 3))])
                scores_panel(1, 1, [(0, AV(0, 2, 0)), (2, AV(0, 2, 1)),
                                    (8, AV(0, 3, 0)), (11, AV(0, 3, 1))],
                             dve_kts={3, 11})
                scores_panel(1, 2, [OP(0, tt) for tt in range(qt_n)]
                             + [(12, AV(1, 0, 0)), (14, AV(1, 0, 1))],
                             dve_kts={3, 7, 11, 14})
                scores_panel(1, 3, [(0, AV(1, 1, 0)), (2, AV(1, 1, 1)),
                                    (6, AV(1, 2, 0)), (9, AV(1, 2, 1))],
                             dve_kts={3, 7, 11, 14})
                # tail: both PV groups of the last panel accumulate
                # kt-interleaved, so only the kt15 matmuls trail the final
                # exp; then divide -> transpose -> out-proj per q-tile
                h, fg, hh = 3, 1, 1
                vc = slice(h * (D + 1), (h + 1) * (D + 1))
                pt = pts[(1, 3)]
                avg = [psav.tile([128, 4, 65], F32, tag="av",
                                 name=f"avt_{g}") for g in range(2)]
                for kt in range(kt_n):
                    for g in range(2):
                        for j in range(4):
                            qt = g * 4 + j
                            nc.tensor.matmul(
                                avg[g][:, j, :], pt[:, kt, ts(qt, 128)],
                                vh_sb[:, kt, vc],
                                start=(kt == 0 and j == 0),
                                stop=(kt == kt_n - 1 and j == 3),
                                skip_group_check=True,
                            )
                for g in range(2):
                    rc4 = stage.tile([128, 4], F32, tag="rc4")
                    nc.vector.reciprocal(rc4, avg[g][:, :, 64:65])
                    for j in range(4):
                        qt = g * 4 + j
                        nc.vector.tensor_scalar_mul(
                            an[1][(qt, fg)][:, hh * 64:(hh + 1) * 64],
                            avg[g][:, j, 0:64], rc4[:, j:j + 1])
                        g0 = qhw + qt * 128
                        for fg2 in range(2):
                            nc.sync.dma_start(
                                out=aT_sb[:, fg2, g0:g0 + 128],
                                in_=an[1][(qt, fg2)],
                                transpose=True,
                            )
                        u_outproj(1, qt, copy_eng="act")

    nc.compile()
    return nc


def _prep_inputs(x, attention_mask, Wq, bq, Wk, bk, Wv, bv, Wo, bo, s=S):
    kt_n = s // 128
    x = np.asarray(x, dtype=np.float32)
    Wq = np.asarray(Wq, np.float32)
    Wk = np.asarray(Wk, np.float32)
    Wv = np.asarray(Wv, np.float32)
    Wo = np.asarray(Wo, np.float32)
    bqf = np.asarray(bq, np.float32)
    bkf = np.asarray(bk, np.float32)
    bvf = np.asarray(bv, np.float32)
    mask = np.asarray(attention_mask, np.float32).reshape(B, s)

    xT_b = [np.ascontiguousarray(x[b].T.astype(NPBF)) for b in range(B)]
    mk_b = [np.ascontiguousarray(mask[b].reshape(kt_n, 128).T)
            for b in range(B)]
    mk2_b = [np.ascontiguousarray(
        (SCH_B + SCH_A * mask[b]).astype(np.float32).reshape(kt_n, 128).T)
        for b in range(B)]

    in_maps = []
    for c in range(8):
        b = c // 4
        hg = c % 4
        r = slice(hg * F, (hg + 1) * F)
        # [128, KC, 2, 128]: partition p, chunk k -> W row k*128+p
        wqc = np.ascontiguousarray(
            Wq[r].T.astype(NPBF).reshape(KC, 128, 2, 128).transpose(1, 0, 2, 3))
        wkc = np.ascontiguousarray(
            Wk[r].T.astype(NPBF).reshape(KC, 128, 2, 128).transpose(1, 0, 2, 3))
        wvc = np.zeros((H, VW), dtype=NPBF)
        bvh1 = np.empty((1, VW), dtype=np.float32)
        for h in range(HPC):
            rh = slice(hg * F + h * D, hg * F + (h + 1) * D)
            wvc[:, h * (D + 1):h * (D + 1) + D] = Wv[rh].T.astype(NPBF)
            bvh1[0, h * (D + 1):h * (D + 1) + D] = bvf[rh]
            bvh1[0, h * (D + 1) + D] = 1.0
        wvc = np.ascontiguousarray(
            wvc.reshape(KC, 128, VW).transpose(1, 0, 2))
        woc = np.ascontiguousarray(
            Wo[:, r].T.astype(NPBF).reshape(2, 128, H).transpose(1, 0, 2))
        in_maps.append({
            "xT": xT_b[b],
            "wq": wqc,
            "wk": wkc,
            "wv": wvc,
            "bq": np.ascontiguousarray(bqf[r].reshape(2, 128).T),
            "bk": np.ascontiguousarray(bkf[r].reshape(2, 128).T),
            "bvh": np.ascontiguousarray(np.broadcast_to(bvh1, (128, VW))),
            "mk": mk_b[b],
            "mk2": mk2_b[b],
            "wo": woc,
        })
    return in_maps


def kernel(x, attention_mask, Wq, bq, Wk, bk, Wv, bv, Wo, bo, _want_results=False):
    from concourse.bass_utils import run_bass_kernel_spmd

    if "nc" not in _CACHE:
        _CACHE["nc"] = _build_nc()
    nc = _CACHE["nc"]
    in_maps = _prep_inputs(x, attention_mask, Wq, bq, Wk, bk, Wv, bv, Wo, bo)
    res = run_bass_kernel_spmd(nc, in_maps, core_ids=list(range(8)))
    acc = np.zeros((B, S, H), dtype=np.float32)
    for c, r in enumerate(res.results):
        acc[c // 4] += r["out"].astype(np.float32)
    acc += np.asarray(bo, dtype=np.float32)[None, None, :]
    if _want_results:
        return acc, res
    return acc
